# revision 1
# baseline (speedup 1.0000x reference)
"""Causal self-attention (B=4, T=2048, C=1024, H=16) on 8 TRN2 NeuronCores.

Sharding (tensor-parallel over batch x head-group): core c handles batch c//2
and heads [8*(c%2), 8*(c%2)+8). Each core computes its local qkv projections,
per-head causal attention, and a row-sharded output-projection partial in a
single fused Bass/Tile program; the host sums the two head-group partials per
batch (the "all-reduce" of the sharding hint) and adds the bias.

Self-contained: builds the Bass program, shards the full inputs, runs SPMD on
cores 0-7 via concourse.bass_utils.run_bass_kernel_spmd, and unshards.
"""
from contextlib import ExitStack

import numpy as np
import concourse.bass as bass
import concourse.mybir as mybir
import concourse.tile as tile
from concourse import bacc

F32 = mybir.dt.float32
F32R = mybir.dt.float32r
BF16 = mybir.dt.bfloat16
EXP = mybir.ActivationFunctionType.Exp

T = 2048          # tokens
C = 1024          # channels
NH = 8            # local heads
HD = 64           # head dim
CL = NH * HD      # local channels (512)
TJ = T // 512     # 4 q-chunks of 512
KC = T // 128     # 16 k-chunks of 128
SCALE = HD ** -0.5


def build_nc(loop_reps: int | None = None):
    nc = bacc.Bacc("TRN2", target_bir_lowering=False, debug=False)
    xT = nc.declare_dram_parameter("xT", [C, T], F32R, isOutput=False)
    wqk = nc.declare_dram_parameter("wqk", [C, 2 * CL], F32R, isOutput=False)
    wv = nc.declare_dram_parameter("wv", [C, CL], F32R, isOutput=False)
    wp = nc.declare_dram_parameter("wp", [4, 128, C], F32R, isOutput=False)
    idn = nc.declare_dram_parameter("idn", [128, 128], BF16, isOutput=False)
    maskm = nc.declare_dram_parameter("maskm", [128, 128], BF16, isOutput=False)
    onec = nc.declare_dram_parameter("onec", [128, 64], F32R, isOutput=False)
    yout = nc.declare_dram_parameter("yout", [T, C], F32, isOutput=True)

    with ExitStack() as ctx:
        ctx.enter_context(nc.allow_low_precision(
            reason="fp32r stores are rounded PE operands; accumulation stays fp32 in PSUM"))
        tc = ctx.enter_context(tile.TileContext(nc, pool_alloc_mode="queue"))

        # ---- persistent pools ----
        consts = ctx.enter_context(tc.tile_pool(name="consts", bufs=1))
        idn_sb = consts.tile([128, 128], BF16)
        maskm_sb = consts.tile([128, 128], BF16)
        ones_sb = consts.tile([128, 64], F32R)
        nc.sync.dma_start(idn_sb[:], idn[:])
        nc.sync.dma_start(maskm_sb[:], maskm[:])
        nc.sync.dma_start(ones_sb[:], onec[:])

        qk_pool = ctx.enter_context(tc.tile_pool(name="qk_pool", bufs=1))
        qkT = [qk_pool.tile([128, T], F32R, name=f"qkT{fi}") for fi in range(8)]
        vaug_pool = ctx.enter_context(tc.tile_pool(name="vaug_pool", bufs=1))
        vaug = [vaug_pool.tile([128, NH * 65], F32R, name=f"vaug{tt}")
                for tt in range(KC)]
        wp_pool = ctx.enter_context(tc.tile_pool(name="wp_pool", bufs=1))
        wp_sb = [wp_pool.tile([128, C], F32R, name=f"wp{pp}") for pp in range(4)]
        for pp in range(4):
            nc.sync.dma_start(wp_sb[pp][:], wp[pp, :, :])

        loop = tc.For_i(0, loop_reps) if loop_reps is not None else None
        if loop is not None:
            ctx.enter_context(loop)

        # =========== phase 1: qkv projections ===========
        with tc.tile_pool(name="w1", bufs=1) as w1, \
             tc.tile_pool(name="xp", bufs=12) as xp, \
             tc.tile_pool(name="ps1", bufs=6, space="PSUM") as ps1:
            wqk_sb = [w1.tile([128, 2 * CL], F32R, name=f"wqk{ci}") for ci in range(8)]
            wv_sb = [w1.tile([128, CL], F32R, name=f"wv{ci}") for ci in range(8)]
            for ci in range(8):
                nc.sync.dma_start(wqk_sb[ci][:], wqk[ci * 128:(ci + 1) * 128, :])
                nc.sync.dma_start(wv_sb[ci][:], wv[ci * 128:(ci + 1) * 128, :])

            for tj in range(TJ):
                xt = []
                for ci in range(8):
                    t_ = xp.tile([128, 512], F32R, name="xt", tag="xt")
                    nc.sync.dma_start(t_[:], xT[ci * 128:(ci + 1) * 128,
                                                 tj * 512:(tj + 1) * 512])
                    xt.append(t_)
                # q,k features: out [feat 128, tok 512]
                for fi in range(8):
                    ps = ps1.tile([128, 512], F32, name="qkps", tag="qkps")
                    for ci in range(8):
                        nc.tensor.matmul(
                            ps[:],
                            (wqk_sb[ci][:, fi * 128:(fi + 1) * 128]),
                            (xt[ci][:]),
                            start=(ci == 0), stop=(ci == 7))
                    nc.vector.tensor_copy(qkT[fi][:, tj * 512:(tj + 1) * 512], ps[:])
                # v: out [tok 128, vfeat 512] -> vaug strided (65-col groups)
                for ts in range(4):
                    tt = tj * 4 + ts
                    ps = ps1.tile([128, 512], F32, name="vps", tag="qkps")
                    for ci in range(8):
                        nc.tensor.matmul(
                            ps[:],
                            (xt[ci][:, ts * 128:(ts + 1) * 128]),
                            (wv_sb[ci][:]),
                            start=(ci == 0), stop=(ci == 7))
                    va = vaug[tt].rearrange("p (h s) -> p h s", s=65)
                    nc.any.tensor_copy(
                        va[:, :, 0:64],
                        ps.rearrange("p (h s) -> p h s", s=64))
                    nc.sync.dma_start(va[:, :, 64:65], onec[:, 0:8])

        # =========== phase 2: attention + proj ===========
        with tc.tile_pool(name="apool", bufs=6) as apool, \
             tc.tile_pool(name="ysbp", bufs=8) as ysbp, \
             tc.tile_pool(name="recp", bufs=2) as recp, \
             tc.tile_pool(name="osb", bufs=3) as osbp, \
             tc.tile_pool(name="sps", bufs=2, space="PSUM") as sps, \
             tc.tile_pool(name="yps", bufs=2, space="PSUM") as yps, \
             tc.tile_pool(name="bps", bufs=1, space="PSUM") as bps, \
             tc.tile_pool(name="ps3", bufs=1, space="PSUM") as ps3:
            for j in range(TJ):
                yts = []
                for p in range(4):
                    pair = (2 * p, 2 * p + 1)
                    # pair-stacked normalized y: head 2p -> rows 0-63 (direct
                    # DVE write), head 2p+1 -> rows 64-127 (via SBUF-SBUF DMA
                    # partition remap) so proj runs full-K=128 matmuls.
                    yt = ysbp.tile([128, 512], F32R, name="yt", tag="yt")
                    yts.append(yt)
                    att = {}
                    # ---- scores (transposed): sT[k, q] + mask + exp ----
                    # K=64 head-pair matmuls are interleaved h0,h1,h0,h1 so
                    # the two heads run concurrently in disjoint PE row groups
                    # (h even -> rows 0-63, h odd -> rows 64-127).
                    for kcg in range(2 * (j + 1)):
                        sp = {h: sps.tile([128, 1024], F32, name="sp", tag="sp")
                              for h in pair}
                        for u in range(2):
                            kc = 2 * kcg + u
                            d = max(0, (kc - 4 * j) * 128)
                            for h in pair:
                                base = (h % 2) * 64
                                ksl = qkT[4 + h // 2][base:base + 64,
                                                     kc * 128:(kc + 1) * 128]
                                qsl = qkT[h // 2][base:base + 64,
                                                  j * 512 + d:(j + 1) * 512]
                                nc.tensor.matmul(
                                    sp[h][:, u * 512 + d:(u + 1) * 512],
                                    (ksl), (qsl),
                                    start=True, stop=(kc < 4 * j),
                                    skip_group_check=True)
                            if kc >= 4 * j:  # diagonal blocks: add mask
                                for h in pair:
                                    nc.tensor.matmul(
                                        sp[h][:, u * 512 + d:u * 512 + d + 128],
                                        idn_sb[:], maskm_sb[:],
                                        start=False, stop=True,
                                        skip_group_check=True)
                        for h in pair:
                            at = apool.tile([128, 1024], F32R, name="at", tag="at")
                            nc.scalar.activation(at[:], sp[h][:], EXP, scale=SCALE)
                            att[(h, kcg)] = at
                    # ---- PV (+denominator via ones column) ----
                    for h in pair:
                        yp = yps.tile([128, 512], F32, name="yp", tag="yp")
                        for kcg in range(2 * (j + 1)):
                            for u in range(2):
                                kc = 2 * kcg + u
                                d = max(0, (kc - 4 * j) * 128)
                                nc.tensor.matmul(
                                    yp[0:65, d:512],
                                    (vaug[kc][:, h * 65:h * 65 + 65]),
                                    (att[(h, kcg)][:, u * 512 + d:(u + 1) * 512]),
                                    start=(kc == 0), stop=(kc == 4 * j + 3),
                                    skip_group_check=True)
                        # ---- normalize: reciprocal of the denominator row,
                        # PE-broadcast it across 64 partitions, scale y ----
                        rc = recp.tile([128, 512], F32R, name="rc", tag="rc")
                        nc.vector.reciprocal(rc[64:65, :], yp[64:65, :])
                        bp = bps.tile([64, 512], F32, name="bp", tag="bp")
                        nc.tensor.matmul(
                            bp[:], (ones_sb[64:65, 0:64]), (rc[64:65, :]),
                            start=True, stop=True)
                        yraw = recp.tile([64, 512], F32, name="yraw", tag="yraw")
                        nc.any.tensor_copy(yraw[:], yp[0:64, :])
                        if h % 2 == 0:
                            nc.vector.tensor_mul(yt[0:64, :], yraw[:], bp[:])
                        else:
                            ytmp = recp.tile([64, 512], F32R, name="ytmp",
                                             tag="ytmp")
                            nc.vector.tensor_mul(ytmp[:], yraw[:], bp[:])
                            nc.sync.dma_start(yt[64:128, :], ytmp[:])
                # ---- proj for this token block ----
                for ts in range(4):
                    for co in range(2):
                        ps = ps3.tile([128, 512], F32, name="pps", tag="pps")
                        for pp in range(4):
                            nc.tensor.matmul(
                                ps[:],
                                (yts[pp][:, ts * 128:(ts + 1) * 128]),
                                (wp_sb[pp][:, co * 512:(co + 1) * 512]),
                                start=(pp == 0), stop=(pp == 3))
                        ot = osbp.tile([128, 512], F32, name="ot", tag="ot")
                        nc.any.tensor_copy(ot[:], ps[:])
                        nc.sync.dma_start(
                            yout[(j * 4 + ts) * 128:(j * 4 + ts + 1) * 128,
                                 co * 512:(co + 1) * 512], ot[:])

    nc.compile()
    return nc


# ---------------- host-side sharding ----------------

def shard_inputs(x, w_qkv, w_proj):
    """Full inputs -> list of 8 per-core input maps."""
    import ml_dtypes
    idn = np.eye(128, dtype=ml_dtypes.bfloat16)
    r = np.arange(128)
    maskm = np.where(r[:, None] > r[None, :], -1e9, 0.0).astype(ml_dtypes.bfloat16)
    in_maps = []
    for core in range(8):
        b, g = core // 2, core % 2
        sl = slice(g * CL, (g + 1) * CL)
        in_maps.append(dict(
            xT=np.ascontiguousarray(x[b].T),
            wqk=np.ascontiguousarray(
                np.concatenate([w_qkv[:, sl], w_qkv[:, C + g * CL:C + (g + 1) * CL]],
                               axis=1)),
            wv=np.ascontiguousarray(w_qkv[:, 2 * C + g * CL:2 * C + (g + 1) * CL]),
            wp=np.ascontiguousarray(w_proj[sl, :].reshape(4, 128, C)),
            idn=idn, maskm=maskm, onec=np.ones((128, 64), np.float32),
        ))
    return in_maps


def unshard_output(results, b_proj):
    """Per-core partial [T, C] projections -> full [B, T, C] output."""
    out = np.empty((4, T, C), dtype=np.float32)
    for b in range(4):
        out[b] = results[2 * b]["yout"] + results[2 * b + 1]["yout"]
    out += b_proj[None, None, :]
    return out


_CACHE = {}


def kernel(x, w_qkv, w_proj, b_proj):
    from concourse.bass_utils import run_bass_kernel_spmd
    if "nc" not in _CACHE:
        _CACHE["nc"] = build_nc()
    nc = _CACHE["nc"]
    in_maps = shard_inputs(np.asarray(x, np.float32),
                           np.asarray(w_qkv, np.float32),
                           np.asarray(w_proj, np.float32))
    res = run_bass_kernel_spmd(nc, in_maps, core_ids=list(range(8)))
    return unshard_output(res.results, np.asarray(b_proj, np.float32))



# revision 2
# speedup vs baseline: 1.1608x; 1.1608x over previous
"""v4: interleaved qkv slices + bf16 x. Causal self-attention (B=4, T=2048, C=1024, H=16) on 8 TRN2 NeuronCores.

Sharding (tensor-parallel over batch x head-group): core c handles batch c//2
and heads [8*(c%2), 8*(c%2)+8). Host sums the two head-group partials per
batch and adds the bias.

v3 on top of v2 (fp8-DoubleRow PV with residual correction):
  - Interleaved schedule: the QKV projection is computed per 512-token chunk
    tj, immediately followed by the attention block j=tj (causality only
    needs K/V chunks <= tj). The PE's projection work for chunk j+1 then
    overlaps the ScalarE softmax of block j instead of serializing.
  - qkT stored bf16 (same PE rate, halves SBUF) so phase-1 and phase-2
    pools can coexist.
  - ones-slots of the v stacks initialized once outside the rep loop;
  - combine DMAs (corr/dn/ytmp) dispatched from the ACT hardware DGE queue
    to unload the SP queue; output tiles merged to [128,1024] per DMA.
  - per pair the odd head is processed first so its SBUF partition-remap
    DMA overlaps the even head's combine.
"""
from contextlib import ExitStack

import numpy as np
import concourse.bass as bass
import concourse.mybir as mybir
import concourse.tile as tile
from concourse import bacc

F32 = mybir.dt.float32
F32R = mybir.dt.float32r
BF16 = mybir.dt.bfloat16
F8 = mybir.dt.float8e4
EXP = mybir.ActivationFunctionType.Exp
DR = mybir.MatmulPerfMode.DoubleRow
MUL = mybir.AluOpType.mult
ADD = mybir.AluOpType.add

T = 2048          # tokens
C = 1024          # channels
NH = 8            # local heads
HD = 64           # head dim
CL = NH * HD      # local channels (512)
TJ = T // 512     # 4 q-chunks of 512
KC = T // 128     # 16 k-chunks of 128
SCALE = HD ** -0.5
BIAS = -3.0       # exp bias for fp8 att storage (cancels in normalization)


def build_nc(loop_reps: int | None = None):
    nc = bacc.Bacc("TRN2", target_bir_lowering=False, debug=False)
    xT = nc.declare_dram_parameter("xT", [C, T], BF16, isOutput=False)
    wqk = nc.declare_dram_parameter("wqk", [C, 2 * CL], BF16, isOutput=False)
    wv = nc.declare_dram_parameter("wv", [C, CL], BF16, isOutput=False)
    wp = nc.declare_dram_parameter("wp", [4, 128, C], F32R, isOutput=False)
    idn = nc.declare_dram_parameter("idn", [128, 128], BF16, isOutput=False)
    maskm = nc.declare_dram_parameter("maskm", [128, 128], BF16, isOutput=False)
    onec = nc.declare_dram_parameter("onec", [128, 64], F32R, isOutput=False)
    one8 = nc.declare_dram_parameter("one8", [128, 8], F8, isOutput=False)
    mskc = nc.declare_dram_parameter("mskc", [64, 1], F32, isOutput=False)
    yout = nc.declare_dram_parameter("yout", [T, C], F32, isOutput=True)

    with ExitStack() as ctx:
        ctx.enter_context(nc.allow_low_precision(
            reason="fp8 PV with residual correction; bf16 scores; fp32r elsewhere"))
        tc = ctx.enter_context(tile.TileContext(nc, pool_alloc_mode="queue"))

        # ---- persistent pools ----
        consts = ctx.enter_context(tc.tile_pool(name="consts", bufs=1))
        idn_sb = consts.tile([128, 128], BF16)
        maskm_sb = consts.tile([128, 128], BF16)
        msk_sb = consts.tile([64, 1], F32)
        bias_sb = consts.tile([128, 1], F32)
        nc.sync.dma_start(idn_sb[:], idn[:])
        nc.sync.dma_start(maskm_sb[:], maskm[:])
        nc.sync.dma_start(msk_sb[:], mskc[:])
        nc.gpsimd.memset(bias_sb[:], BIAS)

        qk_pool = ctx.enter_context(tc.tile_pool(name="qk_pool", bufs=1))
        qkT = [qk_pool.tile([128, T], BF16, name=f"qkT{fi}") for fi in range(8)]
        v0_pool = ctx.enter_context(tc.tile_pool(name="v0_pool", bufs=1))
        vaug0 = [v0_pool.tile([128, NH * 65], F32R, name=f"vaug0{tt}")
                 for tt in range(4)]
        vdr_pool = ctx.enter_context(tc.tile_pool(name="vdr_pool", bufs=1))
        vdr = [vdr_pool.tile([128, 2, NH, 128], F8, name=f"vdr{g}")
               for g in range(KC // 2)]
        wp_pool = ctx.enter_context(tc.tile_pool(name="wp_pool", bufs=1))
        wp_sb = [wp_pool.tile([128, C], F32R, name=f"wp{pp}") for pp in range(4)]
        for pp in range(4):
            nc.sync.dma_start(wp_sb[pp][:], wp[pp, :, :])
        # ones slots are never overwritten by the per-rep v writes: init once
        for g in range(KC // 2):
            for ko in range(2):
                nc.sync.dma_start(vdr[g][:, ko, :, 127:128], one8[:, 0:8])
        for tt in range(4):
            va = vaug0[tt].rearrange("p (h s) -> p h s", s=65)
            nc.sync.dma_start(va[:, :, 64:65], onec[:, 0:8])

        w1 = ctx.enter_context(tc.tile_pool(name="w1", bufs=1))
        wqk_sb = [w1.tile([128, 2 * CL], BF16, name=f"wqk{ci}") for ci in range(8)]
        wv_sb = [w1.tile([128, CL], BF16, name=f"wv{ci}") for ci in range(8)]
        for ci in range(8):
            nc.sync.dma_start(wqk_sb[ci][:], wqk[ci * 128:(ci + 1) * 128, :])
            nc.sync.dma_start(wv_sb[ci][:], wv[ci * 128:(ci + 1) * 128, :])

        loop = tc.For_i(0, loop_reps) if loop_reps is not None else None
        if loop is not None:
            ctx.enter_context(loop)

        with tc.tile_pool(name="xp", bufs=16) as xp, \
             tc.tile_pool(name="ps1", bufs=2, space="PSUM") as ps1, \
             tc.tile_pool(name="ap0", bufs=2) as ap0, \
             tc.tile_pool(name="ap8", bufs=6) as ap8, \
             tc.tile_pool(name="ysbp", bufs=6) as ysbp, \
             tc.tile_pool(name="cmb", bufs=3) as cmb, \
             tc.tile_pool(name="osb", bufs=2) as osbp, \
             tc.tile_pool(name="sps", bufs=2, space="PSUM") as sps, \
             tc.tile_pool(name="yps", bufs=2, space="PSUM") as yps:
            def load_x(tj):
                xt = []
                for ci in range(8):
                    t_ = xp.tile([128, 512], BF16, name="xt", tag="xt")
                    nc.sync.dma_start(t_[:], xT[ci * 128:(ci + 1) * 128,
                                                 tj * 512:(tj + 1) * 512])
                    xt.append(t_)
                return xt

            def qkv_slice(tj, xt, sl):
                """Emit qkv work slice sl (0..3) for token chunk tj."""
                for fi in (2 * sl, 2 * sl + 1):
                    ps = ps1.tile([128, 512], F32, name="qkps", tag="qkps")
                    for ci in range(8):
                        nc.tensor.matmul(
                            ps[:],
                            (wqk_sb[ci][:, fi * 128:(fi + 1) * 128]),
                            (xt[ci][:]),
                            start=(ci == 0), stop=(ci == 7))
                    nc.vector.tensor_copy(qkT[fi][:, tj * 512:(tj + 1) * 512], ps[:])
                ts = sl
                tt = tj * 4 + ts
                ps = ps1.tile([128, 512], F32, name="vps", tag="qkps")
                for ci in range(8):
                    nc.tensor.matmul(
                        ps[:],
                        (xt[ci][:, ts * 128:(ts + 1) * 128]),
                        (wv_sb[ci][:]),
                        start=(ci == 0), stop=(ci == 7))
                ps3 = ps.rearrange("p (h s) -> p h s", s=64)
                g, ko = tt // 2, tt % 2
                vg = vdr[g]
                nc.vector.tensor_copy(vg[:, ko, :, 0:64], ps3[:])
                nc.vector.tensor_sub(vg[:, ko, :, 64:127],
                                     ps3[:, :, 0:63], vg[:, ko, :, 0:63])
                if tt < 4:
                    va = vaug0[tt].rearrange("p (h s) -> p h s", s=65)
                    nc.vector.tensor_copy(va[:, :, 0:64], ps3[:])

            # chunk 0 computed up front; chunk j+1 interleaved into block j
            xt_cur = load_x(0)
            for sl in range(4):
                qkv_slice(0, xt_cur, sl)
            for j in range(TJ):
                if j < TJ - 1:
                    xt_nxt = load_x(j + 1)
                yts = []
                for p in range(4):
                    pair = (2 * p, 2 * p + 1)
                    yt = ysbp.tile([128, 512], F32R, name="yt", tag="yt")
                    yts.append(yt)
                    att = {}
                    for kcg in range(2 * (j + 1)):
                        sp = {h: sps.tile([128, 1024], F32, name="sp", tag="sp")
                              for h in pair}
                        for u in range(2):
                            kc = 2 * kcg + u
                            d = max(0, (kc - 4 * j) * 128)
                            for h in pair:
                                base = (h % 2) * 64
                                ksl = qkT[4 + h // 2][base:base + 64,
                                                     kc * 128:(kc + 1) * 128]
                                qsl = qkT[h // 2][base:base + 64,
                                                  j * 512 + d:(j + 1) * 512]
                                nc.tensor.matmul(
                                    sp[h][:, u * 512 + d:(u + 1) * 512],
                                    (ksl), (qsl),
                                    start=True, stop=(kc < 4 * j),
                                    skip_group_check=True)
                            if kc >= 4 * j:
                                for h in pair:
                                    nc.tensor.matmul(
                                        sp[h][:, u * 512 + d:u * 512 + d + 128],
                                        idn_sb[:], maskm_sb[:],
                                        start=False, stop=True,
                                        skip_group_check=True)
                        lo = 256 if (j > 0 and kcg == 2 * j + 1) else 0
                        for h in pair:
                            if j == 0:
                                at = ap0.tile([128, 1024], F32R, name="at0",
                                              tag="at0")
                                nc.scalar.activation(at[:], sp[h][:], EXP,
                                                     scale=SCALE)
                            else:
                                at = ap8.tile([128, 1024], F8, name="at8",
                                              tag="at8")
                                nc.scalar.activation(
                                    at[:, lo:1024], sp[h][:, lo:1024], EXP,
                                    scale=SCALE, bias=bias_sb[:])
                            att[(h, kcg)] = at
                    # PV + combine, odd head first so its remap DMA overlaps
                    for h in (pair[1], pair[0]):
                        yp = yps.tile([128, 512], F32, name="yp", tag="yp")
                        if j == 0:
                            for kcg in range(2):
                                for u in range(2):
                                    kc = 2 * kcg + u
                                    d = kc * 128
                                    nc.tensor.matmul(
                                        yp[0:65, d:512],
                                        (vaug0[kc][:, h * 65:h * 65 + 65]),
                                        (att[(h, kcg)][:, u * 512 + d:(u + 1) * 512]),
                                        start=(kc == 0), stop=(kc == 3),
                                        skip_group_check=True)
                        else:
                            for kcg in range(2 * (j + 1)):
                                at3 = att[(h, kcg)].rearrange(
                                    "p (ko q) -> p ko q", q=512)
                                kc0, kc1 = 2 * kcg, 2 * kcg + 1
                                d0 = max(0, (kc0 - 4 * j) * 128)
                                d1 = max(0, (kc1 - 4 * j) * 128)
                                if d1 > d0:
                                    nc.tensor.matmul(
                                        yp[:, d0:d1],
                                        vdr[kcg][:, 0, h, :],
                                        at3[:, 0, d0:d1],
                                        start=False, stop=False,
                                        skip_group_check=True)
                                nc.tensor.matmul(
                                    yp[:, d1:512],
                                    vdr[kcg][:, :, h, :],
                                    at3[:, :, d1:512],
                                    start=(kcg == 0), stop=(kcg == 2 * j + 1),
                                    perf_mode=DR, skip_group_check=True)
                        # ---- combine + normalize ----
                        dn = cmb.tile([1, 512], F32, name="dn", tag="dn")
                        rc = cmb.tile([1, 512], F32, name="rc", tag="rc")
                        bp = cmb.tile([64, 512], F32, name="bp", tag="bp")
                        if j == 0:
                            sc = cmb.tile([128, 512], F32, name="sc0", tag="sc0")
                            nc.vector.tensor_copy(sc[64:65, :], yp[64:65, :])
                            nc.scalar.dma_start(dn[:], sc[64:65, :])
                            nc.vector.reciprocal(rc[:], dn[:])
                            nc.gpsimd.partition_broadcast(bp[:], rc[:])
                            if h % 2 == 0:
                                nc.vector.tensor_mul(yt[0:64, :], yp[0:64, :], bp[:])
                            else:
                                ytmp = cmb.tile([64, 512], F32R, name="ytmp",
                                                tag="ytmp")
                                nc.vector.tensor_mul(ytmp[:], yp[0:64, :], bp[:])
                                nc.scalar.dma_start(yt[64:128, :], ytmp[:])
                        else:
                            sc = cmb.tile([128, 512], F32, name="sc", tag="sc0")
                            nc.vector.tensor_copy(sc[64:128, :], yp[64:128, :])
                            corr = cmb.tile([64, 512], F32, name="corr", tag="corr")
                            nc.scalar.dma_start(corr[:], sc[64:128, :])
                            nc.scalar.dma_start(dn[:], sc[127:128, :])
                            nc.vector.reciprocal(rc[:], dn[:])
                            nc.gpsimd.partition_broadcast(bp[:], rc[:])
                            tsum = cmb.tile([64, 512], F32, name="tsum", tag="tsum")
                            nc.vector.scalar_tensor_tensor(
                                tsum[:], corr[:], msk_sb[:], yp[0:64, :],
                                op0=MUL, op1=ADD)
                            if h % 2 == 0:
                                nc.vector.tensor_mul(yt[0:64, :], tsum[:], bp[:])
                            else:
                                ytmp = cmb.tile([64, 512], F32R, name="ytmp",
                                                tag="ytmp")
                                nc.vector.tensor_mul(ytmp[:], tsum[:], bp[:])
                                nc.scalar.dma_start(yt[64:128, :], ytmp[:])
                    # interleaved qkv slice for the next token chunk
                    if j < TJ - 1:
                        qkv_slice(j + 1, xt_nxt, p)
                # ---- proj for this token block ----
                for ts in range(4):
                    ot = osbp.tile([128, 1024], F32, name="ot", tag="ot")
                    for co in range(2):
                        ps = yps.tile([128, 512], F32, name="pps", tag="yp")
                        for pp in range(4):
                            nc.tensor.matmul(
                                ps[:],
                                (yts[pp][:, ts * 128:(ts + 1) * 128]),
                                (wp_sb[pp][:, co * 512:(co + 1) * 512]),
                                start=(pp == 0), stop=(pp == 3))
                        nc.vector.tensor_copy(ot[:, co * 512:(co + 1) * 512], ps[:])
                    nc.sync.dma_start(
                        yout[(j * 4 + ts) * 128:(j * 4 + ts + 1) * 128, :], ot[:])

    nc.compile()
    return nc


# ---------------- host-side sharding ----------------

def shard_inputs(x, w_qkv, w_proj):
    """Full inputs -> list of 8 per-core input maps."""
    import ml_dtypes
    idn = np.eye(128, dtype=ml_dtypes.bfloat16)
    r = np.arange(128)
    maskm = np.where(r[:, None] > r[None, :], -1e9, 0.0).astype(ml_dtypes.bfloat16)
    one8 = np.ones((128, 8), ml_dtypes.float8_e4m3)
    msk = np.ones((64, 1), np.float32)
    msk[63] = 0.0
    in_maps = []
    for core in range(8):
        b, g = core // 2, core % 2
        sl = slice(g * CL, (g + 1) * CL)
        in_maps.append(dict(
            xT=np.ascontiguousarray(x[b].T).astype(ml_dtypes.bfloat16),
            wqk=np.ascontiguousarray(
                np.concatenate([w_qkv[:, sl], w_qkv[:, C + g * CL:C + (g + 1) * CL]],
                               axis=1)).astype(ml_dtypes.bfloat16),
            wv=np.ascontiguousarray(w_qkv[:, 2 * C + g * CL:2 * C + (g + 1) * CL]).astype(ml_dtypes.bfloat16),
            wp=np.ascontiguousarray(w_proj[sl, :].reshape(4, 128, C)),
            idn=idn, maskm=maskm, onec=np.ones((128, 64), np.float32),
            one8=one8, mskc=msk,
        ))
    return in_maps


def unshard_output(results, b_proj):
    """Per-core partial [T, C] projections -> full [B, T, C] output."""
    out = np.empty((4, T, C), dtype=np.float32)
    for b in range(4):
        out[b] = results[2 * b]["yout"] + results[2 * b + 1]["yout"]
    out += b_proj[None, None, :]
    return out


_CACHE = {}


def kernel(x, w_qkv, w_proj, b_proj):
    from concourse.bass_utils import run_bass_kernel_spmd
    if "nc" not in _CACHE:
        _CACHE["nc"] = build_nc()
    nc = _CACHE["nc"]
    in_maps = shard_inputs(np.asarray(x, np.float32),
                           np.asarray(w_qkv, np.float32),
                           np.asarray(w_proj, np.float32))
    res = run_bass_kernel_spmd(nc, in_maps, core_ids=list(range(8)))
    return unshard_output(res.results, np.asarray(b_proj, np.float32))
